# revision 1
# baseline (speedup 1.0000x reference)
"""Trainium2 Bass kernel for GNN message passing:
    out[i] = sum_{e: dst[e]==i} x[src[e]]     (x: [N, 64] f32, edge_index: [2, E] int)

Strategy (graph-partitioned node sharding, 8 cores):
  * Host sorts edges by destination and shards the destination-node space
    across the 8 cores (N/8 nodes per core, x replicated). Each core's
    128-node destination tiles are permuted so heavy tiles align across
    cores (minimizes union padding; host un-permutes rows at the end).
  * x is repacked as [N, 128] bf16 rows: [bf16(x) | bf16(x - bf16(x))]
    (hi|lo split): one 256 B-row gather fetches both halves, one bf16
    matmul per chunk processes both, and they are summed at evacuation —
    ~1e-5 relative accuracy at bf16 PE speed.
  * Edges are grouped per (supertile of 8 dst tiles, source block of 25000
    rows — int16-safe for dma_gather) into contiguous runs, padded only to
    the 128-edge chunk size. Chunks may straddle destination-tile
    boundaries; every (chunk, tile) pair present in ANY core gets a matmul
    slot, and per-core local-dst streams mask foreign edges with -1.
  * Per core, per chunk: dma_gather (GPSIMD) fetches packed rows; VectorE
    builds [128,128] bf16 one-hots (fused 4 slots per tensor_tensor
    is_equal against a replicated iota); TensorE accumulates
    psum[tile] += onehot.T @ msgs (one PSUM bank per live tile);
    ScalarE+VectorE merge hi+lo into SBUF staging at supertile end.
  * Each core stores its padded [N/8, 64] f32 slice with one DMA; the host
    un-permutes tile rows and concatenates. No collectives.
"""

import numpy as np
import ml_dtypes

import concourse.bacc as bacc
import concourse.bass as bass
import concourse.mybir as mybir
import concourse.tile as tile
from concourse.bass_utils import run_bass_kernel_spmd

P = 128
F32 = mybir.dt.float32
BF16 = mybir.dt.bfloat16
I16 = mybir.dt.int16
I32 = mybir.dt.int32
BF = ml_dtypes.bfloat16

# Full-problem constants (hardcoded per harness contract).
N_NODES = 100000
DIM = 64
N_CORES = 8
SRC_BLOCK = 25000        # int16-safe source block
CHUNKS_PER_CALL = 48     # max chunks per dma_gather call; split packets
SUPERTILE = 8            # dst tiles per supertile (<= 8 PSUM banks live)
SINGLE_PACKET = False    # single_packet caps at 64 ring descriptors


def _prep(edge_index, n_nodes, n_cores, block, w, stile=SUPERTILE):
    npc = n_nodes // n_cores
    tiles = -(-npc // P)
    nblocks = -(-n_nodes // block)
    n_super = -(-tiles // stile)

    dst = np.asarray(edge_index[0]).astype(np.int64)
    src = np.asarray(edge_index[1]).astype(np.int64)

    k_of = dst // npc
    t_of = (dst - k_of * npc) // P
    b_of = src // block
    seg = (k_of * tiles + t_of) * nblocks + b_of
    order = np.argsort(seg, kind="stable")
    dst_s = dst[order]
    src_s = src[order]
    seg_s = seg[order]

    counts0 = np.bincount(
        seg_s, minlength=n_cores * tiles * nblocks
    ).reshape(n_cores, tiles, nblocks)
    # global start offset of each (core, true tile, block) bucket
    all_starts = np.concatenate(
        [[0], np.cumsum(counts0.ravel())]
    )

    # tile -> slot permutation per core (align heavy tiles across cores)
    perm = np.argsort(-counts0.sum(axis=2), axis=1, kind="stable")  # [cores, tiles]
    counts = np.take_along_axis(counts0, perm[:, :, None], axis=1)  # [cores,slot,b]

    # ---- runs: (supertile, block) -> concatenated slot buckets, pad to 128
    chunk_block = []
    chunk_super = []
    run_meta = []  # (s, b, chunk0, nch, ends_k [cores, nts], ts)
    for s in range(n_super):
        ts = list(range(s * stile, min((s + 1) * stile, tiles)))
        for b in range(nblocks):
            c_kt = counts[:, ts, b]                      # [cores, nts]
            run_max = int(c_kt.sum(axis=1).max())
            if run_max == 0:
                continue
            nch = -(-run_max // P)
            chunk0 = len(chunk_block)
            chunk_block += [b] * nch
            chunk_super += [s] * nch
            run_meta.append((s, b, chunk0, nch, np.cumsum(c_kt, axis=1), ts))
    ch = len(chunk_block)
    chunk_block = np.array(chunk_block)
    chunk_super = np.array(chunk_super)

    # ---- matmul slots: union over cores of tiles present in each chunk
    mm_chunk = []
    mm_tile = []
    for s, b, chunk0, nch, ends_k, ts in run_meta:
        starts_k = ends_k - counts[:, ts, b]
        for ci_local in range(nch):
            a0, a1 = ci_local * P, (ci_local + 1) * P
            present = ((starts_k < a1) & (ends_k > a0)).any(axis=0)
            for j in np.nonzero(present)[0]:
                mm_chunk.append(chunk0 + ci_local)
                mm_tile.append(ts[j])
    nslots = len(mm_chunk)
    mm_chunk = np.array(mm_chunk)
    mm_tile = np.array(mm_tile)

    mm_first = np.zeros(nslots, dtype=bool)
    mm_last = np.zeros(nslots, dtype=bool)
    seen = set()
    for i in range(nslots):
        t = int(mm_tile[i])
        if t not in seen:
            seen.add(t)
            mm_first[i] = True
    seen = set()
    for i in range(nslots - 1, -1, -1):
        t = int(mm_tile[i])
        if t not in seen:
            seen.add(t)
            mm_last[i] = True
    tile_has = np.zeros(tiles, dtype=bool)
    if nslots:
        tile_has[np.unique(mm_tile)] = True

    # ---- calls: split each run into <= w chunk pieces (same block)
    calls = []  # (block, c0, csize, slot0, nslots_call)
    for s, b, chunk0, nch, ends_k, ts in run_meta:
        start = chunk0
        while start < chunk0 + nch:
            csize = min(w, chunk0 + nch - start)
            s0 = int(np.searchsorted(mm_chunk, start))
            s1 = int(np.searchsorted(mm_chunk, start + csize))
            calls.append((b, start, csize, s0, s1 - s0))
            start += csize
    max_slots_call = max(c[4] for c in calls)

    # ---- per-core streams
    idx_flat = np.zeros((n_cores, ch * P), np.int16)
    ldst_slots = np.full((n_cores, nslots, P), -1.0, BF)
    for k in range(n_cores):
        for s, b, chunk0, nch, ends_k, ts in run_meta:
            pieces_src = []
            pieces_ldst = []
            pieces_slot = []
            for j, t in enumerate(ts):
                cnt = int(counts[k, t, b])
                if cnt == 0:
                    continue
                tt = int(perm[k, t])
                g0 = int(all_starts[(k * tiles + tt) * nblocks + b])
                pieces_src.append(src_s[g0 : g0 + cnt] - b * block)
                pieces_ldst.append(dst_s[g0 : g0 + cnt] - (k * npc + tt * P))
                pieces_slot.append(np.full(cnt, t, np.int64))
            if not pieces_src:
                continue
            esrc = np.concatenate(pieces_src).astype(np.int16)
            eldst = np.concatenate(pieces_ldst)
            eslot = np.concatenate(pieces_slot)
            n_e = esrc.shape[0]
            base = chunk0 * P
            idx_flat[k, base : base + n_e] = esrc
            s0 = int(np.searchsorted(mm_chunk, chunk0))
            s1 = int(np.searchsorted(mm_chunk, chunk0 + nch))
            for i in range(s0, s1):
                ci_local = int(mm_chunk[i]) - chunk0
                t = int(mm_tile[i])
                a0 = ci_local * P
                a1 = min(a0 + P, n_e)
                if a0 >= n_e:
                    continue
                m = eslot[a0:a1] == t
                if not m.any():
                    continue
                col = ldst_slots[k, i]
                col[: a1 - a0][m] = eldst[a0:a1][m].astype(BF)

    idx_all = np.ascontiguousarray(
        np.tile(idx_flat.reshape(n_cores, ch * 8, 16).transpose(0, 2, 1), (1, 8, 1))
    )
    ldst_all = np.ascontiguousarray(ldst_slots.transpose(0, 2, 1))  # [cores,P,nslots]

    return dict(
        npc=npc,
        tiles=tiles,
        nblocks=nblocks,
        n_super=n_super,
        stile=stile,
        ch=ch,
        nslots=nslots,
        calls=calls,
        max_slots_call=max_slots_call,
        chunk_super=chunk_super,
        mm_chunk=mm_chunk,
        mm_tile=mm_tile,
        mm_first=mm_first,
        mm_last=mm_last,
        tile_has_chunks=tile_has,
        idx=idx_all,
        ldst=ldst_all,
        perm=perm,
    )


def _pack_x(x):
    """[N, D] f32 -> [N, 2D] bf16 rows: [hi | lo]."""
    x = np.asarray(x, np.float32)
    hi = x.astype(BF)
    lo = (x - hi.astype(np.float32)).astype(BF)
    return np.ascontiguousarray(np.concatenate([hi, lo], axis=1))


def _build(n_nodes, dim, block, w, sched):
    tiles = sched["tiles"]
    stile = sched["stile"]
    n_super = sched["n_super"]
    ch = sched["ch"]
    nslots = sched["nslots"]
    calls = sched["calls"]
    msc = sched["max_slots_call"]
    chunk_super = sched["chunk_super"]
    mm_chunk = sched["mm_chunk"]
    mm_tile = sched["mm_tile"]
    mm_first = sched["mm_first"]
    mm_last = sched["mm_last"]
    tile_has = sched["tile_has_chunks"]
    out_pad = tiles * P
    elem = 2 * dim  # packed bf16 row length

    nc = bacc.Bacc("TRN2", target_bir_lowering=False, debug=False)
    x_t = nc.dram_tensor("xpack", [n_nodes, elem], BF16, kind="ExternalInput")
    idx_t = nc.dram_tensor("idx", [P, ch * 8], I16, kind="ExternalInput")
    ldst_t = nc.dram_tensor("ldst", [P, nslots], BF16, kind="ExternalInput")
    out_t = nc.dram_tensor("out", [out_pad, dim], F32, kind="ExternalOutput")

    with tile.TileContext(nc) as tc:
        with (
            tc.tile_pool(name="const", bufs=1) as const_pool,
            tc.tile_pool(name="meta", bufs=4) as meta_pool,
            tc.tile_pool(name="gather", bufs=3) as gather_pool,
            tc.tile_pool(name="oh", bufs=8) as oh_pool,
            tc.tile_pool(name="stage", bufs=1) as stage_pool,
            tc.tile_pool(name="psum", bufs=8, space="PSUM") as psum_pool,
        ):
            iota_i = const_pool.tile([P, 4 * P], I32)
            nc.gpsimd.iota(
                iota_i[:], pattern=[[0, 4], [1, P]], base=0, channel_multiplier=0
            )
            iota_b = const_pool.tile([P, 4 * P], BF16)
            nc.vector.tensor_copy(iota_b[:], iota_i[:])

            stage = stage_pool.tile([P, tiles * dim], F32)
            nc.vector.memset(stage[:], 0.0)

            call_idx = 0
            psums = {}
            for s in range(n_super):
                ts = list(range(s * stile, min((s + 1) * stile, tiles)))
                while call_idx < len(calls):
                    b, c0, csize, s0, nsc = calls[call_idx]
                    if int(chunk_super[c0]) != s:
                        break
                    call_idx += 1
                    idx_tile = meta_pool.tile([P, w * 8], I16, tag="idx")
                    nc.sync.dma_start(
                        idx_tile[:, : csize * 8],
                        idx_t[:, c0 * 8 : (c0 + csize) * 8],
                    )
                    ldst_tile = meta_pool.tile([P, msc], BF16, tag="ldst")
                    if nsc:
                        nc.sync.dma_start(
                            ldst_tile[:, :nsc], ldst_t[:, s0 : s0 + nsc]
                        )
                    msgs = gather_pool.tile([P, w, elem], BF16)
                    nc.gpsimd.dma_gather(
                        out_ap=msgs[:, :csize, :],
                        in_ap=x_t[b * block : min((b + 1) * block, n_nodes), :],
                        idxs_ap=idx_tile[:, : csize * 8],
                        num_idxs=csize * P,
                        num_idxs_reg=csize * P,
                        elem_size=elem,
                        single_packet=SINGLE_PACKET,
                    )
                    for j0 in range(0, nsc, 4):
                        g = min(4, nsc - j0)
                        onehot = oh_pool.tile([P, 4 * P], BF16, name="oh", tag="oh")
                        lt = ldst_tile[:, j0 : j0 + g]
                        lt_b = bass.AP(lt.tensor, lt.offset, lt.ap + [[0, P]])
                        nc.vector.tensor_tensor(
                            out=onehot[:, : g * P].rearrange(
                                "p (g q) -> p g q", q=P
                            ),
                            in0=iota_b[:, : g * P].rearrange(
                                "p (g q) -> p g q", q=P
                            ),
                            in1=lt_b,
                            op=mybir.AluOpType.is_equal,
                        )
                        for jj in range(g):
                            si = s0 + j0 + jj
                            t = int(mm_tile[si])
                            cin = int(mm_chunk[si]) - c0
                            if mm_first[si]:
                                psums[t] = psum_pool.tile(
                                    [P, elem], F32, tag="ps", name=f"ps{t}"
                                )
                            nc.tensor.matmul(
                                psums[t][:, :],
                                lhsT=onehot[:, jj * P : (jj + 1) * P],
                                rhs=msgs[:, cin, :],
                                start=bool(mm_first[si]),
                                stop=bool(mm_last[si]),
                            )
                # evacuate: stage[:, t*dim:+dim] = psum_hi + psum_lo
                for t in ts:
                    if not tile_has[t]:
                        continue
                    ps = psums.pop(t)
                    nc.scalar.copy(stage[:, t * dim : (t + 1) * dim], ps[:, :dim])
                    nc.vector.tensor_tensor(
                        out=stage[:, t * dim : (t + 1) * dim],
                        in0=stage[:, t * dim : (t + 1) * dim],
                        in1=ps[:, dim:],
                        op=mybir.AluOpType.add,
                    )

            out_view = out_t[:, :].rearrange("(t p) d -> p t d", p=P)
            nc.sync.dma_start(out_view, stage[:])

    nc.compile()
    return nc


def _run(x, edge_index, n_nodes, dim, n_cores, block, w, **run_kwargs):
    sched = _prep(edge_index, n_nodes, n_cores, block, w)
    xp = _pack_x(x)
    nc = _build(n_nodes, dim, block, w, sched)
    in_maps = [
        {"xpack": xp, "idx": sched["idx"][k], "ldst": sched["ldst"][k]}
        for k in range(n_cores)
    ]
    res = run_bass_kernel_spmd(
        nc, in_maps, core_ids=list(range(n_cores)), **run_kwargs
    )
    npc = sched["npc"]
    tiles = sched["tiles"]
    perm = sched["perm"]
    parts = []
    for k in range(n_cores):
        r = res.results[k]["out"].reshape(tiles, P, -1)
        inv = np.empty(tiles, np.int64)
        inv[perm[k]] = np.arange(tiles)
        parts.append(r[inv].reshape(tiles * P, -1)[:npc])
    out = np.concatenate(parts, axis=0)
    return out, res


def kernel(x, edge_index):
    out, _ = _run(
        x, edge_index, N_NODES, DIM, N_CORES, SRC_BLOCK, CHUNKS_PER_CALL
    )
    return out



# revision 4
# speedup vs baseline: 1.9707x; 1.9707x over previous
"""Trainium2 Bass kernel for GNN message passing:
    out[i] = sum_{e: dst[e]==i} x[src[e]]     (x: [N, 64] f32, edge_index: [2, E] int)

Strategy (graph-partitioned node sharding, 8 cores):
  * Host sorts edges by destination and shards the destination-node space
    across the 8 cores (N/8 nodes per core, x replicated). Each core's
    128-node destination tiles are permuted so heavy tiles align across
    cores (minimizes union padding; host un-permutes rows at the end).
  * x is repacked as [N, 128] bf16 rows: [bf16(x) | bf16(x - bf16(x))]
    (hi|lo split): one 256 B-row gather fetches both halves, one bf16
    matmul per chunk processes both, and they are summed at evacuation —
    ~1e-5 relative accuracy at bf16 PE speed.
  * Edges are grouped per (supertile of 8 dst tiles, source block of 25000
    rows — int16-safe for dma_gather) into contiguous runs, padded only to
    the 128-edge chunk size. Chunks may straddle destination-tile
    boundaries; every (chunk, tile) pair present in ANY core gets a matmul
    slot, and per-core local-dst streams mask foreign edges with -1.
  * Per core, per chunk: dma_gather (GPSIMD) fetches packed rows; VectorE
    builds [128,128] bf16 one-hots (fused 4 slots per tensor_tensor
    is_equal against a replicated iota); TensorE accumulates
    psum[tile] += onehot.T @ msgs (one PSUM bank per live tile);
    ScalarE+VectorE merge hi+lo into SBUF staging at supertile end.
  * Each core stores its padded [N/8, 64] f32 slice with one DMA; the host
    un-permutes tile rows and concatenates. No collectives.
"""

import numpy as np
import ml_dtypes

import concourse.bacc as bacc
import concourse.bass as bass
import concourse.mybir as mybir
import concourse.tile as tile
from concourse.bass_utils import run_bass_kernel_spmd

P = 128
F32 = mybir.dt.float32
BF16 = mybir.dt.bfloat16
I16 = mybir.dt.int16
I32 = mybir.dt.int32
BF = ml_dtypes.bfloat16

# Full-problem constants (hardcoded per harness contract).
N_NODES = 100000
DIM = 64
N_CORES = 8
SRC_BLOCK = 25000        # int16-safe source block
CHUNKS_PER_CALL = 48     # max chunks per dma_gather call; split packets
SUPERTILE = 8            # dst tiles per supertile (<= 8 PSUM banks live)
SINGLE_PACKET = False    # single_packet caps at 64 ring descriptors


def _prep(edge_index, n_nodes, n_cores, block, w, stile=SUPERTILE):
    npc = n_nodes // n_cores
    tiles = -(-npc // P)
    nblocks = -(-n_nodes // block)
    n_super = -(-tiles // stile)

    dst = np.asarray(edge_index[0]).astype(np.int64)
    src = np.asarray(edge_index[1]).astype(np.int64)

    k_of = dst // npc
    t_of = (dst - k_of * npc) // P
    b_of = src // block
    seg = (k_of * tiles + t_of) * nblocks + b_of
    order = np.argsort(seg, kind="stable")
    dst_s = dst[order]
    src_s = src[order]
    seg_s = seg[order]

    counts0 = np.bincount(
        seg_s, minlength=n_cores * tiles * nblocks
    ).reshape(n_cores, tiles, nblocks)
    # global start offset of each (core, true tile, block) bucket
    all_starts = np.concatenate(
        [[0], np.cumsum(counts0.ravel())]
    )

    # tile -> slot permutation per core (align heavy tiles across cores)
    perm = np.argsort(-counts0.sum(axis=2), axis=1, kind="stable")  # [cores, tiles]
    counts = np.take_along_axis(counts0, perm[:, :, None], axis=1)  # [cores,slot,b]

    # ---- runs: (supertile, block) -> concatenated slot buckets, pad to 128
    chunk_block = []
    chunk_super = []
    run_meta = []  # (s, b, chunk0, nch, ends_k [cores, nts], ts)
    for s in range(n_super):
        ts = list(range(s * stile, min((s + 1) * stile, tiles)))
        for b in range(nblocks):
            c_kt = counts[:, ts, b]                      # [cores, nts]
            run_max = int(c_kt.sum(axis=1).max())
            if run_max == 0:
                continue
            nch = -(-run_max // P)
            chunk0 = len(chunk_block)
            chunk_block += [b] * nch
            chunk_super += [s] * nch
            run_meta.append((s, b, chunk0, nch, np.cumsum(c_kt, axis=1), ts))
    ch = len(chunk_block)
    chunk_block = np.array(chunk_block)
    chunk_super = np.array(chunk_super)

    # ---- matmul slots: union over cores of tiles present in each chunk
    mm_chunk = []
    mm_tile = []
    for s, b, chunk0, nch, ends_k, ts in run_meta:
        starts_k = ends_k - counts[:, ts, b]
        for ci_local in range(nch):
            a0, a1 = ci_local * P, (ci_local + 1) * P
            present = ((starts_k < a1) & (ends_k > a0)).any(axis=0)
            for j in np.nonzero(present)[0]:
                mm_chunk.append(chunk0 + ci_local)
                mm_tile.append(ts[j])
    nslots = len(mm_chunk)
    mm_chunk = np.array(mm_chunk)
    mm_tile = np.array(mm_tile)

    mm_first = np.zeros(nslots, dtype=bool)
    mm_last = np.zeros(nslots, dtype=bool)
    seen = set()
    for i in range(nslots):
        t = int(mm_tile[i])
        if t not in seen:
            seen.add(t)
            mm_first[i] = True
    seen = set()
    for i in range(nslots - 1, -1, -1):
        t = int(mm_tile[i])
        if t not in seen:
            seen.add(t)
            mm_last[i] = True
    tile_has = np.zeros(tiles, dtype=bool)
    if nslots:
        tile_has[np.unique(mm_tile)] = True

    # ---- calls: split each run into <= w chunk pieces (same block)
    calls = []  # (block, c0, csize, slot0, nslots_call)
    for s, b, chunk0, nch, ends_k, ts in run_meta:
        start = chunk0
        while start < chunk0 + nch:
            csize = min(w, chunk0 + nch - start)
            s0 = int(np.searchsorted(mm_chunk, start))
            s1 = int(np.searchsorted(mm_chunk, start + csize))
            calls.append((b, start, csize, s0, s1 - s0))
            start += csize
    max_slots_call = max(c[4] for c in calls)

    # ---- per-core streams
    idx_flat = np.zeros((n_cores, ch * P), np.int16)
    ldst_slots = np.full((n_cores, nslots, P), -1.0, BF)
    for k in range(n_cores):
        for s, b, chunk0, nch, ends_k, ts in run_meta:
            pieces_src = []
            pieces_ldst = []
            pieces_slot = []
            for j, t in enumerate(ts):
                cnt = int(counts[k, t, b])
                if cnt == 0:
                    continue
                tt = int(perm[k, t])
                g0 = int(all_starts[(k * tiles + tt) * nblocks + b])
                pieces_src.append(src_s[g0 : g0 + cnt] - b * block)
                pieces_ldst.append(dst_s[g0 : g0 + cnt] - (k * npc + tt * P))
                pieces_slot.append(np.full(cnt, t, np.int64))
            if not pieces_src:
                continue
            esrc = np.concatenate(pieces_src).astype(np.int16)
            eldst = np.concatenate(pieces_ldst)
            eslot = np.concatenate(pieces_slot)
            n_e = esrc.shape[0]
            base = chunk0 * P
            idx_flat[k, base : base + n_e] = esrc
            s0 = int(np.searchsorted(mm_chunk, chunk0))
            s1 = int(np.searchsorted(mm_chunk, chunk0 + nch))
            for i in range(s0, s1):
                ci_local = int(mm_chunk[i]) - chunk0
                t = int(mm_tile[i])
                a0 = ci_local * P
                a1 = min(a0 + P, n_e)
                if a0 >= n_e:
                    continue
                m = eslot[a0:a1] == t
                if not m.any():
                    continue
                col = ldst_slots[k, i]
                col[: a1 - a0][m] = eldst[a0:a1][m].astype(BF)

    idx_all = np.ascontiguousarray(
        np.tile(idx_flat.reshape(n_cores, ch * 8, 16).transpose(0, 2, 1), (1, 8, 1))
    )
    ldst_all = np.ascontiguousarray(ldst_slots.transpose(0, 2, 1))  # [cores,P,nslots]

    return dict(
        npc=npc,
        tiles=tiles,
        nblocks=nblocks,
        n_super=n_super,
        stile=stile,
        ch=ch,
        nslots=nslots,
        calls=calls,
        max_slots_call=max_slots_call,
        chunk_super=chunk_super,
        mm_chunk=mm_chunk,
        mm_tile=mm_tile,
        mm_first=mm_first,
        mm_last=mm_last,
        tile_has_chunks=tile_has,
        idx=idx_all,
        ldst=ldst_all,
        perm=perm,
    )


def _pack_x(x):
    """[N, D] f32 -> [N, 2D] bf16 rows: [hi | lo]."""
    x = np.asarray(x, np.float32)
    hi = x.astype(BF)
    lo = (x - hi.astype(np.float32)).astype(BF)
    return np.ascontiguousarray(np.concatenate([hi, lo], axis=1))


def _build(n_nodes, dim, block, w, sched):
    tiles = sched["tiles"]
    stile = sched["stile"]
    n_super = sched["n_super"]
    ch = sched["ch"]
    nslots = sched["nslots"]
    calls = sched["calls"]
    msc = sched["max_slots_call"]
    chunk_super = sched["chunk_super"]
    mm_chunk = sched["mm_chunk"]
    mm_tile = sched["mm_tile"]
    mm_first = sched["mm_first"]
    mm_last = sched["mm_last"]
    tile_has = sched["tile_has_chunks"]
    out_pad = tiles * P
    elem = 2 * dim  # packed bf16 row length

    nc = bacc.Bacc(
        "TRN2", target_bir_lowering=False, debug=False, num_swdge_queues=4
    )
    x_t = nc.dram_tensor("xpack", [n_nodes, elem], BF16, kind="ExternalInput")
    idx_t = nc.dram_tensor("idx", [P, ch * 8], I16, kind="ExternalInput")
    ldst_t = nc.dram_tensor("ldst", [P, nslots], BF16, kind="ExternalInput")
    out_t = nc.dram_tensor("out", [out_pad, dim], F32, kind="ExternalOutput")

    with tile.TileContext(nc) as tc:
        with (
            tc.tile_pool(name="const", bufs=1) as const_pool,
            tc.tile_pool(name="meta", bufs=4) as meta_pool,
            tc.tile_pool(name="gather", bufs=3) as gather_pool,
            tc.tile_pool(name="oh", bufs=8) as oh_pool,
            tc.tile_pool(name="stage", bufs=1) as stage_pool,
            tc.tile_pool(name="psum", bufs=8, space="PSUM") as psum_pool,
        ):
            iota_i = const_pool.tile([P, 4 * P], I32)
            nc.gpsimd.iota(
                iota_i[:], pattern=[[0, 4], [1, P]], base=0, channel_multiplier=0
            )
            iota_b = const_pool.tile([P, 4 * P], BF16)
            nc.vector.tensor_copy(iota_b[:], iota_i[:])

            stage = stage_pool.tile([P, tiles * dim], F32)
            nc.vector.memset(stage[:], 0.0)

            call_idx = 0
            gather_q = 0
            psums = {}
            for s in range(n_super):
                ts = list(range(s * stile, min((s + 1) * stile, tiles)))
                while call_idx < len(calls):
                    b, c0, csize, s0, nsc = calls[call_idx]
                    if int(chunk_super[c0]) != s:
                        break
                    call_idx += 1
                    idx_tile = meta_pool.tile([P, w * 8], I16, tag="idx")
                    nc.sync.dma_start(
                        idx_tile[:, : csize * 8],
                        idx_t[:, c0 * 8 : (c0 + csize) * 8],
                    )
                    ldst_tile = meta_pool.tile([P, msc], BF16, tag="ldst")
                    if nsc:
                        nc.sync.dma_start(
                            ldst_tile[:, :nsc], ldst_t[:, s0 : s0 + nsc]
                        )
                    msgs = gather_pool.tile([P, w, elem], BF16)
                    nc.gpsimd.dma_gather(
                        out_ap=msgs[:, :csize, :],
                        in_ap=x_t[b * block : min((b + 1) * block, n_nodes), :],
                        idxs_ap=idx_tile[:, : csize * 8],
                        num_idxs=csize * P,
                        num_idxs_reg=csize * P,
                        elem_size=elem,
                        single_packet=SINGLE_PACKET,
                        queue_num=gather_q,
                    )
                    gather_q = (gather_q + 1) % 4
                    for j0 in range(0, nsc, 4):
                        g = min(4, nsc - j0)
                        onehot = oh_pool.tile([P, 4 * P], BF16, name="oh", tag="oh")
                        lt = ldst_tile[:, j0 : j0 + g]
                        lt_b = bass.AP(lt.tensor, lt.offset, lt.ap + [[0, P]])
                        nc.vector.tensor_tensor(
                            out=onehot[:, : g * P].rearrange(
                                "p (g q) -> p g q", q=P
                            ),
                            in0=iota_b[:, : g * P].rearrange(
                                "p (g q) -> p g q", q=P
                            ),
                            in1=lt_b,
                            op=mybir.AluOpType.is_equal,
                        )
                        for jj in range(g):
                            si = s0 + j0 + jj
                            t = int(mm_tile[si])
                            cin = int(mm_chunk[si]) - c0
                            if mm_first[si]:
                                psums[t] = psum_pool.tile(
                                    [P, elem], F32, tag="ps", name=f"ps{t}"
                                )
                            nc.tensor.matmul(
                                psums[t][:, :],
                                lhsT=onehot[:, jj * P : (jj + 1) * P],
                                rhs=msgs[:, cin, :],
                                start=bool(mm_first[si]),
                                stop=bool(mm_last[si]),
                            )
                # evacuate: stage[:, t*dim:+dim] = psum_hi + psum_lo
                for t in ts:
                    if not tile_has[t]:
                        continue
                    ps = psums.pop(t)
                    nc.scalar.copy(stage[:, t * dim : (t + 1) * dim], ps[:, :dim])
                    nc.vector.tensor_tensor(
                        out=stage[:, t * dim : (t + 1) * dim],
                        in0=stage[:, t * dim : (t + 1) * dim],
                        in1=ps[:, dim:],
                        op=mybir.AluOpType.add,
                    )

            out_view = out_t[:, :].rearrange("(t p) d -> p t d", p=P)
            nc.sync.dma_start(out_view, stage[:])

    nc.compile()
    return nc


def _run(x, edge_index, n_nodes, dim, n_cores, block, w, **run_kwargs):
    sched = _prep(edge_index, n_nodes, n_cores, block, w)
    xp = _pack_x(x)
    nc = _build(n_nodes, dim, block, w, sched)
    in_maps = [
        {"xpack": xp, "idx": sched["idx"][k], "ldst": sched["ldst"][k]}
        for k in range(n_cores)
    ]
    res = run_bass_kernel_spmd(
        nc, in_maps, core_ids=list(range(n_cores)), **run_kwargs
    )
    npc = sched["npc"]
    tiles = sched["tiles"]
    perm = sched["perm"]
    parts = []
    for k in range(n_cores):
        r = res.results[k]["out"].reshape(tiles, P, -1)
        inv = np.empty(tiles, np.int64)
        inv[perm[k]] = np.arange(tiles)
        parts.append(r[inv].reshape(tiles * P, -1)[:npc])
    out = np.concatenate(parts, axis=0)
    return out, res


def kernel(x, edge_index):
    out, _ = _run(
        x, edge_index, N_NODES, DIM, N_CORES, SRC_BLOCK, CHUNKS_PER_CALL
    )
    return out



# revision 7
# speedup vs baseline: 2.5198x; 1.2786x over previous
"""Trainium2 Bass kernel for GNN message passing:
    out[i] = sum_{e: dst[e]==i} x[src[e]]     (x: [N, 64] f32, edge_index: [2, E] int)

Strategy (graph-partitioned node sharding, 8 cores):
  * Host sorts edges by destination and shards the destination-node space
    across the 8 cores (N/8 nodes per core, x replicated). Each core's
    128-node destination tiles are permuted so heavy tiles align across
    cores (minimizes union padding; host un-permutes rows at the end).
  * x is repacked as [N, 128] bf16 rows: [bf16(x) | bf16(x - bf16(x))]
    (hi|lo split): one 256 B-row gather fetches both halves, one bf16
    matmul per chunk processes both, and they are summed at evacuation —
    ~1e-5 relative accuracy at bf16 PE speed.
  * Edges are grouped per (supertile of 8 dst tiles, source block of 25000
    rows — int16-safe for dma_gather) into contiguous runs, padded only to
    the 128-edge chunk size. Chunks may straddle destination-tile
    boundaries; every (chunk, tile) pair present in ANY core gets a matmul
    slot, and per-core local-dst streams mask foreign edges with -1.
  * Per core, per chunk: dma_gather (GPSIMD) fetches packed rows; VectorE
    builds [128,128] bf16 one-hots (fused 4 slots per tensor_tensor
    is_equal against a replicated iota); TensorE accumulates
    psum[tile] += onehot.T @ msgs (one PSUM bank per live tile);
    ScalarE+VectorE merge hi+lo into SBUF staging at supertile end.
  * Each core stores its padded [N/8, 64] f32 slice with one DMA; the host
    un-permutes tile rows and concatenates. No collectives.
"""

import numpy as np
import ml_dtypes

import concourse.bacc as bacc
import concourse.bass as bass
import concourse.mybir as mybir
import concourse.tile as tile
from concourse.bass_utils import run_bass_kernel_spmd

P = 128
F32 = mybir.dt.float32
BF16 = mybir.dt.bfloat16
I16 = mybir.dt.int16
I32 = mybir.dt.int32
BF = ml_dtypes.bfloat16

# Full-problem constants (hardcoded per harness contract).
N_NODES = 100000
DIM = 64
N_CORES = 8
SRC_BLOCK = 25000        # int16-safe source block
CHUNKS_PER_CALL = 16     # max chunks per dma_gather call; split packets
SUPERTILE = 8            # dst tiles per supertile (<= 8 PSUM banks live)
SINGLE_PACKET = False    # single_packet caps at 64 ring descriptors


def _prep(edge_index, n_nodes, n_cores, block, w, stile=SUPERTILE):
    npc = n_nodes // n_cores
    tiles = -(-npc // P)
    nblocks = -(-n_nodes // block)
    n_super = -(-tiles // stile)

    dst = np.asarray(edge_index[0]).astype(np.int64)
    src = np.asarray(edge_index[1]).astype(np.int64)

    k_of = dst // npc
    t_of = (dst - k_of * npc) // P
    b_of = src // block
    seg = (k_of * tiles + t_of) * nblocks + b_of
    order = np.argsort(seg, kind="stable")
    dst_s = dst[order]
    src_s = src[order]
    seg_s = seg[order]

    counts0 = np.bincount(
        seg_s, minlength=n_cores * tiles * nblocks
    ).reshape(n_cores, tiles, nblocks)
    # global start offset of each (core, true tile, block) bucket
    all_starts = np.concatenate(
        [[0], np.cumsum(counts0.ravel())]
    )

    # tile -> slot permutation per core (align heavy tiles across cores)
    perm = np.argsort(-counts0.sum(axis=2), axis=1, kind="stable")  # [cores, tiles]
    counts = np.take_along_axis(counts0, perm[:, :, None], axis=1)  # [cores,slot,b]

    # ---- runs: (supertile, block) -> concatenated slot buckets, pad to 128
    chunk_block = []
    chunk_super = []
    run_meta = []  # (s, b, chunk0, nch, ends_k [cores, nts], ts)
    for s in range(n_super):
        ts = list(range(s * stile, min((s + 1) * stile, tiles)))
        for b in range(nblocks):
            c_kt = counts[:, ts, b]                      # [cores, nts]
            run_max = int(c_kt.sum(axis=1).max())
            if run_max == 0:
                continue
            nch = -(-run_max // P)
            chunk0 = len(chunk_block)
            chunk_block += [b] * nch
            chunk_super += [s] * nch
            run_meta.append((s, b, chunk0, nch, np.cumsum(c_kt, axis=1), ts))
    ch = len(chunk_block)
    chunk_block = np.array(chunk_block)
    chunk_super = np.array(chunk_super)

    # ---- matmul slots: union over cores of tiles present in each chunk
    mm_chunk = []
    mm_tile = []
    for s, b, chunk0, nch, ends_k, ts in run_meta:
        starts_k = ends_k - counts[:, ts, b]
        for ci_local in range(nch):
            a0, a1 = ci_local * P, (ci_local + 1) * P
            present = ((starts_k < a1) & (ends_k > a0)).any(axis=0)
            for j in np.nonzero(present)[0]:
                mm_chunk.append(chunk0 + ci_local)
                mm_tile.append(ts[j])
    nslots = len(mm_chunk)
    mm_chunk = np.array(mm_chunk)
    mm_tile = np.array(mm_tile)

    mm_first = np.zeros(nslots, dtype=bool)
    mm_last = np.zeros(nslots, dtype=bool)
    seen = set()
    for i in range(nslots):
        t = int(mm_tile[i])
        if t not in seen:
            seen.add(t)
            mm_first[i] = True
    seen = set()
    for i in range(nslots - 1, -1, -1):
        t = int(mm_tile[i])
        if t not in seen:
            seen.add(t)
            mm_last[i] = True
    tile_has = np.zeros(tiles, dtype=bool)
    if nslots:
        tile_has[np.unique(mm_tile)] = True

    # ---- calls: split each run into <= w chunk pieces (same block)
    calls = []  # (block, c0, csize, slot0, nslots_call)
    for s, b, chunk0, nch, ends_k, ts in run_meta:
        start = chunk0
        while start < chunk0 + nch:
            csize = min(w, chunk0 + nch - start)
            s0 = int(np.searchsorted(mm_chunk, start))
            s1 = int(np.searchsorted(mm_chunk, start + csize))
            calls.append((b, start, csize, s0, s1 - s0))
            start += csize
    max_slots_call = max(c[4] for c in calls)

    # ---- per-core streams
    idx_flat = np.zeros((n_cores, ch * P), np.int16)
    ldst_slots = np.full((n_cores, nslots, P), -1.0, BF)
    for k in range(n_cores):
        for s, b, chunk0, nch, ends_k, ts in run_meta:
            pieces_src = []
            pieces_ldst = []
            pieces_slot = []
            for j, t in enumerate(ts):
                cnt = int(counts[k, t, b])
                if cnt == 0:
                    continue
                tt = int(perm[k, t])
                g0 = int(all_starts[(k * tiles + tt) * nblocks + b])
                pieces_src.append(src_s[g0 : g0 + cnt] - b * block)
                pieces_ldst.append(dst_s[g0 : g0 + cnt] - (k * npc + tt * P))
                pieces_slot.append(np.full(cnt, t, np.int64))
            if not pieces_src:
                continue
            esrc = np.concatenate(pieces_src).astype(np.int16)
            eldst = np.concatenate(pieces_ldst)
            eslot = np.concatenate(pieces_slot)
            n_e = esrc.shape[0]
            base = chunk0 * P
            idx_flat[k, base : base + n_e] = esrc
            s0 = int(np.searchsorted(mm_chunk, chunk0))
            s1 = int(np.searchsorted(mm_chunk, chunk0 + nch))
            for i in range(s0, s1):
                ci_local = int(mm_chunk[i]) - chunk0
                t = int(mm_tile[i])
                a0 = ci_local * P
                a1 = min(a0 + P, n_e)
                if a0 >= n_e:
                    continue
                m = eslot[a0:a1] == t
                if not m.any():
                    continue
                col = ldst_slots[k, i]
                col[: a1 - a0][m] = eldst[a0:a1][m].astype(BF)

    idx_all = np.ascontiguousarray(
        np.tile(idx_flat.reshape(n_cores, ch * 8, 16).transpose(0, 2, 1), (1, 8, 1))
    )
    ldst_all = np.ascontiguousarray(ldst_slots.transpose(0, 2, 1))  # [cores,P,nslots]

    return dict(
        npc=npc,
        tiles=tiles,
        nblocks=nblocks,
        n_super=n_super,
        stile=stile,
        ch=ch,
        nslots=nslots,
        calls=calls,
        max_slots_call=max_slots_call,
        chunk_super=chunk_super,
        mm_chunk=mm_chunk,
        mm_tile=mm_tile,
        mm_first=mm_first,
        mm_last=mm_last,
        tile_has_chunks=tile_has,
        idx=idx_all,
        ldst=ldst_all,
        perm=perm,
    )


def _pack_x(x):
    """[N, D] f32 -> [N, 2D] bf16 rows: [hi | lo]."""
    x = np.asarray(x, np.float32)
    hi = x.astype(BF)
    lo = (x - hi.astype(np.float32)).astype(BF)
    return np.ascontiguousarray(np.concatenate([hi, lo], axis=1))


def _build(n_nodes, dim, block, w, sched):
    tiles = sched["tiles"]
    stile = sched["stile"]
    n_super = sched["n_super"]
    ch = sched["ch"]
    nslots = sched["nslots"]
    calls = sched["calls"]
    msc = sched["max_slots_call"]
    chunk_super = sched["chunk_super"]
    mm_chunk = sched["mm_chunk"]
    mm_tile = sched["mm_tile"]
    mm_first = sched["mm_first"]
    mm_last = sched["mm_last"]
    tile_has = sched["tile_has_chunks"]
    out_pad = tiles * P
    elem = 2 * dim  # packed bf16 row length

    nc = bacc.Bacc(
        "TRN2", target_bir_lowering=False, debug=False, num_swdge_queues=4
    )
    x_t = nc.dram_tensor("xpack", [n_nodes, elem], BF16, kind="ExternalInput")
    idx_t = nc.dram_tensor("idx", [P, ch * 8], I16, kind="ExternalInput")
    ldst_t = nc.dram_tensor("ldst", [P, nslots], BF16, kind="ExternalInput")
    out_t = nc.dram_tensor("out", [out_pad, dim], F32, kind="ExternalOutput")

    with tile.TileContext(nc) as tc:
        with (
            tc.tile_pool(name="const", bufs=1) as const_pool,
            tc.tile_pool(name="meta", bufs=8) as meta_pool,
            tc.tile_pool(name="gather", bufs=8) as gather_pool,
            tc.tile_pool(name="oh", bufs=8) as oh_pool,
            tc.tile_pool(name="stage", bufs=1) as stage_pool,
            tc.tile_pool(name="psum", bufs=8, space="PSUM") as psum_pool,
        ):
            iota_i = const_pool.tile([P, 4 * P], I32)
            nc.gpsimd.iota(
                iota_i[:], pattern=[[0, 4], [1, P]], base=0, channel_multiplier=0
            )
            iota_b = const_pool.tile([P, 4 * P], BF16)
            nc.vector.tensor_copy(iota_b[:], iota_i[:])

            stage = stage_pool.tile([P, tiles * dim], F32)
            nc.vector.memset(stage[:], 0.0)

            call_idx = 0
            gather_q = 0
            psums = {}
            for s in range(n_super):
                ts = list(range(s * stile, min((s + 1) * stile, tiles)))
                while call_idx < len(calls):
                    b, c0, csize, s0, nsc = calls[call_idx]
                    if int(chunk_super[c0]) != s:
                        break
                    call_idx += 1
                    idx_tile = meta_pool.tile([P, w * 8], I16, tag="idx")
                    nc.sync.dma_start(
                        idx_tile[:, : csize * 8],
                        idx_t[:, c0 * 8 : (c0 + csize) * 8],
                    )
                    ldst_tile = meta_pool.tile([P, msc], BF16, tag="ldst")
                    if nsc:
                        nc.sync.dma_start(
                            ldst_tile[:, :nsc], ldst_t[:, s0 : s0 + nsc]
                        )
                    msgs = gather_pool.tile([P, w, elem], BF16)
                    nc.gpsimd.dma_gather(
                        out_ap=msgs[:, :csize, :],
                        in_ap=x_t[b * block : min((b + 1) * block, n_nodes), :],
                        idxs_ap=idx_tile[:, : csize * 8],
                        num_idxs=csize * P,
                        num_idxs_reg=csize * P,
                        elem_size=elem,
                        single_packet=SINGLE_PACKET,
                        queue_num=gather_q,
                    )
                    gather_q = (gather_q + 1) % 4
                    for j0 in range(0, nsc, 4):
                        g = min(4, nsc - j0)
                        onehot = oh_pool.tile([P, 4 * P], BF16, name="oh", tag="oh")
                        lt = ldst_tile[:, j0 : j0 + g]
                        lt_b = bass.AP(lt.tensor, lt.offset, lt.ap + [[0, P]])
                        nc.vector.tensor_tensor(
                            out=onehot[:, : g * P].rearrange(
                                "p (g q) -> p g q", q=P
                            ),
                            in0=iota_b[:, : g * P].rearrange(
                                "p (g q) -> p g q", q=P
                            ),
                            in1=lt_b,
                            op=mybir.AluOpType.is_equal,
                        )
                        for jj in range(g):
                            si = s0 + j0 + jj
                            t = int(mm_tile[si])
                            cin = int(mm_chunk[si]) - c0
                            if mm_first[si]:
                                psums[t] = psum_pool.tile(
                                    [P, elem], F32, tag="ps", name=f"ps{t}"
                                )
                            nc.tensor.matmul(
                                psums[t][:, :],
                                lhsT=onehot[:, jj * P : (jj + 1) * P],
                                rhs=msgs[:, cin, :],
                                start=bool(mm_first[si]),
                                stop=bool(mm_last[si]),
                            )
                # evacuate: stage[:, t*dim:+dim] = psum_hi + psum_lo
                for t in ts:
                    if not tile_has[t]:
                        continue
                    ps = psums.pop(t)
                    nc.scalar.copy(stage[:, t * dim : (t + 1) * dim], ps[:, :dim])
                    nc.vector.tensor_tensor(
                        out=stage[:, t * dim : (t + 1) * dim],
                        in0=stage[:, t * dim : (t + 1) * dim],
                        in1=ps[:, dim:],
                        op=mybir.AluOpType.add,
                    )

            out_view = out_t[:, :].rearrange("(t p) d -> p t d", p=P)
            nc.sync.dma_start(out_view, stage[:])

    nc.compile()
    return nc


def _run(x, edge_index, n_nodes, dim, n_cores, block, w, **run_kwargs):
    sched = _prep(edge_index, n_nodes, n_cores, block, w)
    xp = _pack_x(x)
    nc = _build(n_nodes, dim, block, w, sched)
    in_maps = [
        {"xpack": xp, "idx": sched["idx"][k], "ldst": sched["ldst"][k]}
        for k in range(n_cores)
    ]
    res = run_bass_kernel_spmd(
        nc, in_maps, core_ids=list(range(n_cores)), **run_kwargs
    )
    npc = sched["npc"]
    tiles = sched["tiles"]
    perm = sched["perm"]
    parts = []
    for k in range(n_cores):
        r = res.results[k]["out"].reshape(tiles, P, -1)
        inv = np.empty(tiles, np.int64)
        inv[perm[k]] = np.arange(tiles)
        parts.append(r[inv].reshape(tiles * P, -1)[:npc])
    out = np.concatenate(parts, axis=0)
    return out, res


def kernel(x, edge_index):
    out, _ = _run(
        x, edge_index, N_NODES, DIM, N_CORES, SRC_BLOCK, CHUNKS_PER_CALL
    )
    return out



# revision 8
# speedup vs baseline: 2.9279x; 1.1620x over previous
"""Trainium2 Bass kernel for GNN message passing:
    out[i] = sum_{e: dst[e]==i} x[src[e]]     (x: [N, 64] f32, edge_index: [2, E] int)

Strategy (graph-partitioned node sharding, 8 cores):
  * Host sorts edges by destination and shards the destination-node space
    across the 8 cores (N/8 nodes per core, x replicated). Each core's
    128-node destination tiles are permuted so heavy tiles align across
    cores (minimizes union padding; host un-permutes rows at the end).
  * x is repacked as [N, 128] bf16 rows: [bf16(x) | bf16(x - bf16(x))]
    (hi|lo split): one 256 B-row gather fetches both halves, one bf16
    matmul per chunk processes both, and they are summed at evacuation —
    ~1e-5 relative accuracy at bf16 PE speed.
  * Edges are grouped per (supertile of 8 dst tiles, source block of 25000
    rows — int16-safe for dma_gather) into contiguous runs, padded only to
    the 128-edge chunk size. Chunks may straddle destination-tile
    boundaries; every (chunk, tile) pair present in ANY core gets a matmul
    slot, and per-core local-dst streams mask foreign edges with -1.
  * Per core, per chunk: dma_gather (GPSIMD) fetches packed rows; VectorE
    builds [128,128] bf16 one-hots (fused 4 slots per tensor_tensor
    is_equal against a replicated iota); TensorE accumulates
    psum[tile] += onehot.T @ msgs (one PSUM bank per live tile);
    ScalarE+VectorE merge hi+lo into SBUF staging at supertile end.
  * Each core stores its padded [N/8, 64] f32 slice with one DMA; the host
    un-permutes tile rows and concatenates. No collectives.
"""

import numpy as np
import ml_dtypes

import concourse.bacc as bacc
import concourse.bass as bass
import concourse.mybir as mybir
import concourse.tile as tile
from concourse.bass_utils import run_bass_kernel_spmd

P = 128
F32 = mybir.dt.float32
BF16 = mybir.dt.bfloat16
I16 = mybir.dt.int16
I32 = mybir.dt.int32
BF = ml_dtypes.bfloat16

# Full-problem constants (hardcoded per harness contract).
N_NODES = 100000
DIM = 64
N_CORES = 8
SRC_BLOCK = 25000        # int16-safe source block
CHUNKS_PER_CALL = 8      # max chunks per dma_gather call; split packets
SUPERTILE = 8            # dst tiles per supertile (<= 8 PSUM banks live)
SINGLE_PACKET = False    # single_packet caps at 64 ring descriptors


def _prep(edge_index, n_nodes, n_cores, block, w, stile=SUPERTILE):
    npc = n_nodes // n_cores
    tiles = -(-npc // P)
    nblocks = -(-n_nodes // block)
    n_super = -(-tiles // stile)

    dst = np.asarray(edge_index[0]).astype(np.int64)
    src = np.asarray(edge_index[1]).astype(np.int64)

    k_of = dst // npc
    t_of = (dst - k_of * npc) // P
    b_of = src // block
    seg = (k_of * tiles + t_of) * nblocks + b_of
    order = np.argsort(seg, kind="stable")
    dst_s = dst[order]
    src_s = src[order]
    seg_s = seg[order]

    counts0 = np.bincount(
        seg_s, minlength=n_cores * tiles * nblocks
    ).reshape(n_cores, tiles, nblocks)
    # global start offset of each (core, true tile, block) bucket
    all_starts = np.concatenate(
        [[0], np.cumsum(counts0.ravel())]
    )

    # tile -> slot permutation per core (align heavy tiles across cores)
    perm = np.argsort(-counts0.sum(axis=2), axis=1, kind="stable")  # [cores, tiles]
    counts = np.take_along_axis(counts0, perm[:, :, None], axis=1)  # [cores,slot,b]

    # ---- runs: (supertile, block) -> concatenated slot buckets, pad to 128
    chunk_block = []
    chunk_super = []
    run_meta = []  # (s, b, chunk0, nch, ends_k [cores, nts], ts)
    for s in range(n_super):
        ts = list(range(s * stile, min((s + 1) * stile, tiles)))
        for b in range(nblocks):
            c_kt = counts[:, ts, b]                      # [cores, nts]
            run_max = int(c_kt.sum(axis=1).max())
            if run_max == 0:
                continue
            nch = -(-run_max // P)
            chunk0 = len(chunk_block)
            chunk_block += [b] * nch
            chunk_super += [s] * nch
            run_meta.append((s, b, chunk0, nch, np.cumsum(c_kt, axis=1), ts))
    ch = len(chunk_block)
    chunk_block = np.array(chunk_block)
    chunk_super = np.array(chunk_super)

    # ---- matmul slots: union over cores of tiles present in each chunk
    mm_chunk = []
    mm_tile = []
    for s, b, chunk0, nch, ends_k, ts in run_meta:
        starts_k = ends_k - counts[:, ts, b]
        for ci_local in range(nch):
            a0, a1 = ci_local * P, (ci_local + 1) * P
            present = ((starts_k < a1) & (ends_k > a0)).any(axis=0)
            for j in np.nonzero(present)[0]:
                mm_chunk.append(chunk0 + ci_local)
                mm_tile.append(ts[j])
    nslots = len(mm_chunk)
    mm_chunk = np.array(mm_chunk)
    mm_tile = np.array(mm_tile)

    mm_first = np.zeros(nslots, dtype=bool)
    mm_last = np.zeros(nslots, dtype=bool)
    seen = set()
    for i in range(nslots):
        t = int(mm_tile[i])
        if t not in seen:
            seen.add(t)
            mm_first[i] = True
    seen = set()
    for i in range(nslots - 1, -1, -1):
        t = int(mm_tile[i])
        if t not in seen:
            seen.add(t)
            mm_last[i] = True
    tile_has = np.zeros(tiles, dtype=bool)
    if nslots:
        tile_has[np.unique(mm_tile)] = True

    # ---- calls: split each run into <= w chunk pieces (same block)
    calls = []  # (block, c0, csize, slot0, nslots_call)
    for s, b, chunk0, nch, ends_k, ts in run_meta:
        start = chunk0
        while start < chunk0 + nch:
            csize = min(w, chunk0 + nch - start)
            s0 = int(np.searchsorted(mm_chunk, start))
            s1 = int(np.searchsorted(mm_chunk, start + csize))
            calls.append((b, start, csize, s0, s1 - s0))
            start += csize
    max_slots_call = max(c[4] for c in calls)

    # ---- per-core streams
    idx_flat = np.zeros((n_cores, ch * P), np.int16)
    ldst_slots = np.full((n_cores, nslots, P), -1.0, BF)
    for k in range(n_cores):
        for s, b, chunk0, nch, ends_k, ts in run_meta:
            pieces_src = []
            pieces_ldst = []
            pieces_slot = []
            for j, t in enumerate(ts):
                cnt = int(counts[k, t, b])
                if cnt == 0:
                    continue
                tt = int(perm[k, t])
                g0 = int(all_starts[(k * tiles + tt) * nblocks + b])
                pieces_src.append(src_s[g0 : g0 + cnt] - b * block)
                pieces_ldst.append(dst_s[g0 : g0 + cnt] - (k * npc + tt * P))
                pieces_slot.append(np.full(cnt, t, np.int64))
            if not pieces_src:
                continue
            esrc = np.concatenate(pieces_src).astype(np.int16)
            eldst = np.concatenate(pieces_ldst)
            eslot = np.concatenate(pieces_slot)
            n_e = esrc.shape[0]
            base = chunk0 * P
            idx_flat[k, base : base + n_e] = esrc
            s0 = int(np.searchsorted(mm_chunk, chunk0))
            s1 = int(np.searchsorted(mm_chunk, chunk0 + nch))
            for i in range(s0, s1):
                ci_local = int(mm_chunk[i]) - chunk0
                t = int(mm_tile[i])
                a0 = ci_local * P
                a1 = min(a0 + P, n_e)
                if a0 >= n_e:
                    continue
                m = eslot[a0:a1] == t
                if not m.any():
                    continue
                col = ldst_slots[k, i]
                col[: a1 - a0][m] = eldst[a0:a1][m].astype(BF)

    idx_all = np.ascontiguousarray(
        np.tile(idx_flat.reshape(n_cores, ch * 8, 16).transpose(0, 2, 1), (1, 8, 1))
    )
    ldst_all = np.ascontiguousarray(ldst_slots.transpose(0, 2, 1))  # [cores,P,nslots]

    return dict(
        npc=npc,
        tiles=tiles,
        nblocks=nblocks,
        n_super=n_super,
        stile=stile,
        ch=ch,
        nslots=nslots,
        calls=calls,
        max_slots_call=max_slots_call,
        chunk_super=chunk_super,
        mm_chunk=mm_chunk,
        mm_tile=mm_tile,
        mm_first=mm_first,
        mm_last=mm_last,
        tile_has_chunks=tile_has,
        idx=idx_all,
        ldst=ldst_all,
        perm=perm,
    )


def _pack_x(x):
    """[N, D] f32 -> [N, 2D] bf16 rows: [hi | lo]."""
    x = np.asarray(x, np.float32)
    hi = x.astype(BF)
    lo = (x - hi.astype(np.float32)).astype(BF)
    return np.ascontiguousarray(np.concatenate([hi, lo], axis=1))


def _build(n_nodes, dim, block, w, sched):
    tiles = sched["tiles"]
    stile = sched["stile"]
    n_super = sched["n_super"]
    ch = sched["ch"]
    nslots = sched["nslots"]
    calls = sched["calls"]
    msc = sched["max_slots_call"]
    chunk_super = sched["chunk_super"]
    mm_chunk = sched["mm_chunk"]
    mm_tile = sched["mm_tile"]
    mm_first = sched["mm_first"]
    mm_last = sched["mm_last"]
    tile_has = sched["tile_has_chunks"]
    out_pad = tiles * P
    elem = 2 * dim  # packed bf16 row length

    nc = bacc.Bacc(
        "TRN2", target_bir_lowering=False, debug=False, num_swdge_queues=4
    )
    x_t = nc.dram_tensor("xpack", [n_nodes, elem], BF16, kind="ExternalInput")
    idx_t = nc.dram_tensor("idx", [P, ch * 8], I16, kind="ExternalInput")
    ldst_t = nc.dram_tensor("ldst", [P, nslots], BF16, kind="ExternalInput")
    out_t = nc.dram_tensor("out", [out_pad, dim], F32, kind="ExternalOutput")

    with tile.TileContext(nc) as tc:
        with (
            tc.tile_pool(name="const", bufs=1) as const_pool,
            tc.tile_pool(name="meta", bufs=8) as meta_pool,
            tc.tile_pool(name="gather", bufs=8) as gather_pool,
            tc.tile_pool(name="oh", bufs=8) as oh_pool,
            tc.tile_pool(name="stage", bufs=1) as stage_pool,
            tc.tile_pool(name="psum", bufs=8, space="PSUM") as psum_pool,
        ):
            iota_i = const_pool.tile([P, 4 * P], I32)
            nc.gpsimd.iota(
                iota_i[:], pattern=[[0, 4], [1, P]], base=0, channel_multiplier=0
            )
            iota_b = const_pool.tile([P, 4 * P], BF16)
            nc.vector.tensor_copy(iota_b[:], iota_i[:])

            stage = stage_pool.tile([P, tiles * dim], F32)
            nc.vector.memset(stage[:], 0.0)

            call_idx = 0
            gather_q = 0
            psums = {}
            for s in range(n_super):
                ts = list(range(s * stile, min((s + 1) * stile, tiles)))
                while call_idx < len(calls):
                    b, c0, csize, s0, nsc = calls[call_idx]
                    if int(chunk_super[c0]) != s:
                        break
                    call_idx += 1
                    idx_tile = meta_pool.tile([P, w * 8], I16, tag="idx")
                    nc.sync.dma_start(
                        idx_tile[:, : csize * 8],
                        idx_t[:, c0 * 8 : (c0 + csize) * 8],
                    )
                    ldst_tile = meta_pool.tile([P, msc], BF16, tag="ldst")
                    if nsc:
                        nc.sync.dma_start(
                            ldst_tile[:, :nsc], ldst_t[:, s0 : s0 + nsc]
                        )
                    msgs = gather_pool.tile([P, w, elem], BF16)
                    nc.gpsimd.dma_gather(
                        out_ap=msgs[:, :csize, :],
                        in_ap=x_t[b * block : min((b + 1) * block, n_nodes), :],
                        idxs_ap=idx_tile[:, : csize * 8],
                        num_idxs=csize * P,
                        num_idxs_reg=csize * P,
                        elem_size=elem,
                        single_packet=SINGLE_PACKET,
                        queue_num=gather_q,
                    )
                    gather_q = (gather_q + 1) % 4
                    for j0 in range(0, nsc, 4):
                        g = min(4, nsc - j0)
                        onehot = oh_pool.tile([P, 4 * P], BF16, name="oh", tag="oh")
                        lt = ldst_tile[:, j0 : j0 + g]
                        lt_b = bass.AP(lt.tensor, lt.offset, lt.ap + [[0, P]])
                        nc.vector.tensor_tensor(
                            out=onehot[:, : g * P].rearrange(
                                "p (g q) -> p g q", q=P
                            ),
                            in0=iota_b[:, : g * P].rearrange(
                                "p (g q) -> p g q", q=P
                            ),
                            in1=lt_b,
                            op=mybir.AluOpType.is_equal,
                        )
                        for jj in range(g):
                            si = s0 + j0 + jj
                            t = int(mm_tile[si])
                            cin = int(mm_chunk[si]) - c0
                            if mm_first[si]:
                                psums[t] = psum_pool.tile(
                                    [P, elem], F32, tag="ps", name=f"ps{t}"
                                )
                            nc.tensor.matmul(
                                psums[t][:, :],
                                lhsT=onehot[:, jj * P : (jj + 1) * P],
                                rhs=msgs[:, cin, :],
                                start=bool(mm_first[si]),
                                stop=bool(mm_last[si]),
                            )
                # evacuate: stage[:, t*dim:+dim] = psum_hi + psum_lo
                for t in ts:
                    if not tile_has[t]:
                        continue
                    ps = psums.pop(t)
                    nc.scalar.copy(stage[:, t * dim : (t + 1) * dim], ps[:, :dim])
                    nc.vector.tensor_tensor(
                        out=stage[:, t * dim : (t + 1) * dim],
                        in0=stage[:, t * dim : (t + 1) * dim],
                        in1=ps[:, dim:],
                        op=mybir.AluOpType.add,
                    )

            out_view = out_t[:, :].rearrange("(t p) d -> p t d", p=P)
            nc.sync.dma_start(out_view, stage[:])

    nc.compile()
    return nc


def _run(x, edge_index, n_nodes, dim, n_cores, block, w, **run_kwargs):
    sched = _prep(edge_index, n_nodes, n_cores, block, w)
    xp = _pack_x(x)
    nc = _build(n_nodes, dim, block, w, sched)
    in_maps = [
        {"xpack": xp, "idx": sched["idx"][k], "ldst": sched["ldst"][k]}
        for k in range(n_cores)
    ]
    res = run_bass_kernel_spmd(
        nc, in_maps, core_ids=list(range(n_cores)), **run_kwargs
    )
    npc = sched["npc"]
    tiles = sched["tiles"]
    perm = sched["perm"]
    parts = []
    for k in range(n_cores):
        r = res.results[k]["out"].reshape(tiles, P, -1)
        inv = np.empty(tiles, np.int64)
        inv[perm[k]] = np.arange(tiles)
        parts.append(r[inv].reshape(tiles * P, -1)[:npc])
    out = np.concatenate(parts, axis=0)
    return out, res


def kernel(x, edge_index):
    out, _ = _run(
        x, edge_index, N_NODES, DIM, N_CORES, SRC_BLOCK, CHUNKS_PER_CALL
    )
    return out



# revision 9
# speedup vs baseline: 2.9993x; 1.0244x over previous
"""Trainium2 Bass kernel for GNN message passing:
    out[i] = sum_{e: dst[e]==i} x[src[e]]     (x: [N, 64] f32, edge_index: [2, E] int)

Strategy (node-sharded dst, 8 cores, aligned buckets):
  * Host assigns dst nodes to 8 cores x 100 tiles of 128 slots, balancing
    per-(tile, src-block) edge counts to <= 512 so nearly every bucket is
    exactly 4 chunks of 128 edges; chunk boundaries never straddle tiles,
    so every chunk needs exactly ONE matmul (slots == chunks).
  * x is repacked as [N, 128] bf16 rows [hi | lo] (hi/lo split of f32):
    one 256 B gather per edge feeds one [128,128] bf16 matmul; hi+lo
    columns are merged after PSUM evacuation (~1e-5 relative accuracy).
  * dma_gather (GPSIMD SWDGE) is the bottleneck engine: calls are split
    into 8-chunk pieces round-robined over 4 SWDGE queues (8 Q7 cores).
  * One-hot matrices are precomputed on the HOST and streamed in via the
    Activation-engine HWDGE queue (bf16 [128, ch*128]), freeing VectorE.
  * TensorE: psum[tile] += onehot.T @ msgs, accumulated across the 4 src
    blocks of a supertile of 8 tiles (8 PSUM banks); ScalarE+VectorE merge
    hi+lo into SBUF staging at supertile end; one final DMA stores the
    padded [12800, 64] slice; host un-permutes rows. No collectives.
"""

import numpy as np
import ml_dtypes

import concourse.bacc as bacc
import concourse.bass as bass
import concourse.mybir as mybir
import concourse.tile as tile
from concourse.bass_utils import run_bass_kernel_spmd

P = 128
F32 = mybir.dt.float32
BF16 = mybir.dt.bfloat16
I16 = mybir.dt.int16
BF = ml_dtypes.bfloat16

# Full-problem constants (hardcoded per harness contract).
N_NODES = 100000
DIM = 64
N_CORES = 8
SRC_BLOCK = 25000        # int16-safe source block
CHUNKS_PER_CALL = 8      # chunks per dma_gather call piece
TILES_PC = 100           # dst tiles per core (128 slots each, 12800 padded)
SUPERTILE = 8            # dst tiles with live PSUM banks
N_QUEUES = 4             # SWDGE queues (max 4)
SINGLE_PACKET = False
BUCKET_CAP = 512         # target per-(tile, block) edge count (4 chunks)


def _balance(deg, n_bins, cap, sweeps=10):
    """Assign nodes to n_bins bins (equal node counts) s.t. per-bin,
    per-block degree sums are (mostly) <= cap. Returns bin_of [N]."""
    n, nb = deg.shape
    total = deg.sum(1)
    order = np.argsort(-total, kind="stable")
    idx = np.arange(n)
    rows, cols = idx // n_bins, idx % n_bins
    cols = np.where(rows % 2 == 0, cols, n_bins - 1 - cols)
    bin_of = np.empty(n, np.int32)
    bin_of[order] = cols.astype(np.int32)

    loads = np.zeros((n_bins, nb), np.int64)
    np.add.at(loads, bin_of, deg)
    members = [list(np.where(bin_of == b)[0]) for b in range(n_bins)]

    for _ in range(sweeps):
        viol = np.argwhere(loads > cap)
        if len(viol) == 0:
            break
        for bb, blk in viol:
            tries = 0
            while loads[bb, blk] > cap and tries < 20:
                tries += 1
                mem = np.array(members[bb])
                nsel = mem[np.argmax(deg[mem, blk])]
                tgt = int(np.argmin(loads[:, blk]))
                if tgt == bb:
                    break
                tmem = np.array(members[tgt])
                msel = tmem[np.argmin(deg[tmem, blk])]
                if deg[msel, blk] >= deg[nsel, blk]:
                    break
                delta = deg[nsel] - deg[msel]
                if loads[tgt, blk] + delta[blk] > cap:
                    break
                loads[bb] -= delta
                loads[tgt] += delta
                members[bb].remove(nsel)
                members[bb].append(msel)
                members[tgt].remove(msel)
                members[tgt].append(nsel)
                bin_of[nsel] = tgt
                bin_of[msel] = bb
    return bin_of, loads, members


def _prep(edge_index, n_nodes, n_cores, block, w):
    tiles = TILES_PC
    nblocks = -(-n_nodes // block)
    n_bins = n_cores * tiles
    stile = SUPERTILE
    n_super = -(-tiles // stile)

    dst = np.asarray(edge_index[0]).astype(np.int64)
    src = np.asarray(edge_index[1]).astype(np.int64)
    blk_of = (src // block).astype(np.int64)

    deg = np.zeros((n_nodes, nblocks), np.int32)
    np.add.at(deg, (dst, blk_of), 1)

    bin_of, loads, members = _balance(deg, n_bins, BUCKET_CAP)

    # per-core slot ordering: heavy bins aligned across cores
    slot_of_bin = np.empty(n_bins, np.int64)
    bin_at = np.empty((n_cores, tiles), np.int64)
    for k in range(n_cores):
        bins_k = np.arange(k * tiles, (k + 1) * tiles)
        rank = np.argsort(-loads[bins_k].max(axis=1), kind="stable")
        slot_of_bin[bins_k[rank]] = np.arange(tiles)
        bin_at[k, np.arange(tiles)] = bins_k[rank]

    # chunks per (slot, block): shared across cores
    ld = loads.reshape(n_cores, tiles, nblocks)  # indexed by raw bin
    ld_slot = np.empty_like(ld)
    for k in range(n_cores):
        ld_slot[k] = ld[k][bin_at[k] - k * tiles]
    Q = np.maximum(-(-ld_slot.max(axis=0) // P), 1)  # [tiles, nblocks]

    # node -> (pos within its bin)
    node_pos = np.empty(n_nodes, np.int64)
    node_order = np.full((n_cores, tiles, P), -1, np.int64)
    for b in range(n_bins):
        mem = np.array(members[b], dtype=np.int64)
        node_pos[mem] = np.arange(len(mem))
        k = b // tiles
        s = slot_of_bin[b]
        node_order[k, s, : len(mem)] = mem

    # chunk layout in execution order: supertile -> block -> slot
    chunk_tile = []
    chunk_block = []
    bucket_c0 = np.zeros((tiles, nblocks), np.int64)
    calls = []  # (block, c0, csize, queue)
    for sti in range(n_super):
        ts = list(range(sti * stile, min((sti + 1) * stile, tiles)))
        for b in range(nblocks):
            g0 = len(chunk_tile)
            for s in ts:
                bucket_c0[s, b] = len(chunk_tile)
                chunk_tile += [s] * int(Q[s, b])
                chunk_block += [b] * int(Q[s, b])
            g1 = len(chunk_tile)
            c = g0
            while c < g1:
                csize = min(w, g1 - c)
                calls.append((b, c, csize))
                c += csize
    ch = len(chunk_tile)
    chunk_tile = np.array(chunk_tile)
    chunk_block = np.array(chunk_block)

    # first/last chunk per tile (for PSUM start/stop), scoped per supertile
    mm_first = np.zeros(ch, dtype=bool)
    mm_last = np.zeros(ch, dtype=bool)
    for sti in range(n_super):
        ts = range(sti * stile, min((sti + 1) * stile, tiles))
        for s in ts:
            cs = np.where(chunk_tile == s)[0]
            mm_first[cs[0]] = True
            mm_last[cs[-1]] = True

    # per-core streams
    idx_all = np.zeros((n_cores, P, ch * 8), np.int16)
    oh_all = np.zeros((n_cores, P, ch * P), BF)
    qarange = np.arange(P, dtype=np.int16)
    for k in range(n_cores):
        mask = (bin_of[dst] // tiles) == k
        ek = np.where(mask)[0]
        s_e = slot_of_bin[bin_of[dst[ek]]]
        b_e = blk_of[ek]
        key = s_e * nblocks + b_e
        order = np.argsort(key, kind="stable")
        ek = ek[order]
        s_e = s_e[order]
        b_e = b_e[order]
        # rank within bucket
        key_s = key[order]
        uniq, start_idx = np.unique(key_s, return_index=True)
        rank = np.arange(len(ek)) - np.repeat(start_idx, np.diff(
            np.append(start_idx, len(ek))))
        pos = bucket_c0[s_e, b_e] * P + rank
        idx_flat = np.zeros(ch * P, np.int16)
        idx_flat[pos] = (src[ek] - b_e * block).astype(np.int16)
        ohcol = np.full(ch * P, -1, np.int16)
        ohcol[pos] = node_pos[dst[ek]].astype(np.int16)
        # idx wrap: element j -> [j % 16, j // 16], replicated to 128 parts
        idx_all[k] = np.tile(
            idx_flat.reshape(ch * 8, 16).T, (8, 1)
        )
        # one-hot [p, c*128 + q] = (ohcol[c*128+p] == q)
        oh = (ohcol.reshape(ch, P)[:, :, None] == qarange[None, None, :])
        oh_all[k] = np.ascontiguousarray(
            oh.transpose(1, 0, 2).reshape(P, ch * P)
        ).astype(BF)

    return dict(
        tiles=tiles,
        nblocks=nblocks,
        n_super=n_super,
        stile=stile,
        ch=ch,
        calls=calls,
        chunk_tile=chunk_tile,
        mm_first=mm_first,
        mm_last=mm_last,
        idx=idx_all,
        oh=oh_all,
        node_order=node_order,
        Q=Q,
    )


def _pack_x(x):
    """[N, D] f32 -> [N, 2D] bf16 rows: [hi | lo]."""
    x = np.asarray(x, np.float32)
    hi = x.astype(BF)
    lo = (x - hi.astype(np.float32)).astype(BF)
    return np.ascontiguousarray(np.concatenate([hi, lo], axis=1))


def _build(n_nodes, dim, block, w, sched):
    tiles = sched["tiles"]
    stile = sched["stile"]
    n_super = sched["n_super"]
    ch = sched["ch"]
    calls = sched["calls"]
    chunk_tile = sched["chunk_tile"]
    mm_first = sched["mm_first"]
    mm_last = sched["mm_last"]
    out_pad = tiles * P
    elem = 2 * dim  # packed bf16 row length

    nc = bacc.Bacc(
        "TRN2", target_bir_lowering=False, debug=False,
        num_swdge_queues=N_QUEUES,
    )
    x_t = nc.dram_tensor("xpack", [n_nodes, elem], BF16, kind="ExternalInput")
    idx_t = nc.dram_tensor("idx", [P, ch * 8], I16, kind="ExternalInput")
    oh_t = nc.dram_tensor("oh", [P, ch * P], BF16, kind="ExternalInput")
    out_t = nc.dram_tensor("out", [out_pad, dim], F32, kind="ExternalOutput")

    with tile.TileContext(nc) as tc:
        with (
            tc.tile_pool(name="meta", bufs=8) as meta_pool,
            tc.tile_pool(name="ohp", bufs=8) as oh_pool,
            tc.tile_pool(name="gather", bufs=8) as gather_pool,
            tc.tile_pool(name="stage", bufs=1) as stage_pool,
            tc.tile_pool(name="psum", bufs=8, space="PSUM") as psum_pool,
        ):
            stage = stage_pool.tile([P, tiles * dim], F32)
            nc.vector.memset(stage[:], 0.0)

            call_idx = 0
            gather_q = 0
            psums = {}
            for sti in range(n_super):
                ts = list(range(sti * stile, min((sti + 1) * stile, tiles)))
                first_c = None
                while call_idx < len(calls):
                    b, c0, csize = calls[call_idx]
                    if chunk_tile[c0] not in ts:
                        break
                    call_idx += 1
                    idx_tile = meta_pool.tile([P, w * 8], I16, tag="idx")
                    nc.sync.dma_start(
                        idx_tile[:, : csize * 8],
                        idx_t[:, c0 * 8 : (c0 + csize) * 8],
                    )
                    oh_tile = oh_pool.tile([P, w, P], BF16, tag="oh")
                    nc.scalar.dma_start(
                        oh_tile[:, :csize, :],
                        oh_t[:, c0 * P : (c0 + csize) * P].rearrange(
                            "p (c q) -> p c q", q=P
                        ),
                    )
                    msgs = gather_pool.tile([P, w, elem], BF16)
                    nc.gpsimd.dma_gather(
                        out_ap=msgs[:, :csize, :],
                        in_ap=x_t[b * block : min((b + 1) * block, n_nodes), :],
                        idxs_ap=idx_tile[:, : csize * 8],
                        num_idxs=csize * P,
                        num_idxs_reg=csize * P,
                        elem_size=elem,
                        single_packet=SINGLE_PACKET,
                        queue_num=gather_q,
                    )
                    gather_q = (gather_q + 1) % N_QUEUES
                    for j in range(csize):
                        c = c0 + j
                        t = int(chunk_tile[c])
                        if mm_first[c]:
                            psums[t] = psum_pool.tile(
                                [P, elem], F32, tag="ps", name=f"ps{t}"
                            )
                        nc.tensor.matmul(
                            psums[t][:, :],
                            lhsT=oh_tile[:, j, :],
                            rhs=msgs[:, j, :],
                            start=bool(mm_first[c]),
                            stop=bool(mm_last[c]),
                        )
                # evacuate: stage[:, t*dim:+dim] = psum_hi + psum_lo
                for t in ts:
                    if t not in psums:
                        continue
                    ps = psums.pop(t)
                    nc.scalar.copy(stage[:, t * dim : (t + 1) * dim], ps[:, :dim])
                    nc.vector.tensor_tensor(
                        out=stage[:, t * dim : (t + 1) * dim],
                        in0=stage[:, t * dim : (t + 1) * dim],
                        in1=ps[:, dim:],
                        op=mybir.AluOpType.add,
                    )

            out_view = out_t[:, :].rearrange("(t p) d -> p t d", p=P)
            nc.sync.dma_start(out_view, stage[:])

    nc.compile()
    return nc


def _run(x, edge_index, n_nodes, dim, n_cores, block, w, **run_kwargs):
    sched = _prep(edge_index, n_nodes, n_cores, block, w)
    xp = _pack_x(x)
    nc = _build(n_nodes, dim, block, w, sched)
    in_maps = [
        {"xpack": xp, "idx": sched["idx"][k], "oh": sched["oh"][k]}
        for k in range(n_cores)
    ]
    res = run_bass_kernel_spmd(
        nc, in_maps, core_ids=list(range(n_cores)), **run_kwargs
    )
    node_order = sched["node_order"]  # [cores, tiles, P]
    out = np.zeros((n_nodes, dim), np.float32)
    for k in range(n_cores):
        r = res.results[k]["out"]  # [tiles*P, dim]
        no = node_order[k].reshape(-1)
        m = no >= 0
        out[no[m]] = r[m]
    return out, res


def kernel(x, edge_index):
    out, _ = _run(
        x, edge_index, N_NODES, DIM, N_CORES, SRC_BLOCK, CHUNKS_PER_CALL
    )
    return out
